# revision 14
# baseline (speedup 1.0000x reference)
"""LocalWindowAttention TRN2 kernel.

Full inputs -> full output. Sharding: 8 cores = batch(4) x seq-half(2).
Each core computes 2048 query positions; k/v halos (128 each side) come
from overlapping the per-core x slice, so no collectives are needed.

Math (per core, matching reference):
  qkv = x @ Wqkv + bqkv  (q pre-scaled by 1/sqrt(1024) via the weights)
  banded attention, window 128, block size 128: query tile e attends key
  tiles e-1, e, e+1 with a static band mask |kpos - qpos| <= 128.
  Softmax without max-subtraction (scores are O(0.1)); invalid keys are
  zeroed AFTER exp via a 0/1 band mask, out-of-sequence keys are zeroed
  via a validity indicator carried as a 65th column of v (which also
  yields the softmax denominator through the PV matmul).
  out = attn_out @ Wout + bout
"""

import sys

import numpy as np

for _p in ("/opt/trn_rl_repo",):
    if _p not in sys.path:
        sys.path.insert(0, _p)

import ml_dtypes  # noqa: E402

import concourse.bass as bass  # noqa: E402
import concourse.mybir as mybir  # noqa: E402
import concourse.tile as tile  # noqa: E402
from concourse import bacc  # noqa: E402
from concourse.bass_utils import run_bass_kernel_spmd  # noqa: E402
from concourse.masks import make_identity  # noqa: E402

F32 = mybir.dt.float32
F32R = mybir.dt.float32r
BF16 = mybir.dt.float16  # attention compute dtype (fp16: 11-bit mantissa, full PE rate)

B, S, D = 4, 4096, 1024
H, DH, W = 16, 64, 128
N_CORES = 8
S_LOC = 2048            # query positions per core
T_Q = S_LOC // W        # 16 query tiles per core
T_EXT = T_Q + 2         # 18 extended tiles (with halo)
S_EXT = T_EXT * W       # 2304
NQK = 2 * D             # q+k projected features
KC = D // 128           # 8 contraction chunks
HP = H // 2             # 8 head pairs
VCOL = DH + 1           # 64 v dims + indicator column

# pos groups for the projection matmuls (>=256 so float32r runs full speed)
POSGROUPS = [(0, 512), (512, 512), (1024, 512), (1536, 512), (2048, 256)]


def _build_nc():
    nc = bacc.Bacc(
        "TRN2",
        target_bir_lowering=False,
        debug=False,
        num_devices=N_CORES,
    )

    xT_d = nc.dram_tensor("xT", [D, S_EXT], F32R, kind="ExternalInput").ap()
    wqkv_d = nc.dram_tensor("wqkv", [D, 3 * D], F32R, kind="ExternalInput").ap()
    bqk_d = nc.dram_tensor("bqk", [128, 16], F32, kind="ExternalInput").ap()
    bvb_d = nc.dram_tensor("bvb", [1, D], F32, kind="ExternalInput").ap()
    wout_d = nc.dram_tensor("wout", [D, D], F32R, kind="ExternalInput").ap()
    boutb_d = nc.dram_tensor("boutb", [1, D], F32, kind="ExternalInput").ap()
    trimask_d = nc.dram_tensor("trimask", [128, 3 * W], BF16, kind="ExternalInput").ap()
    indp_d = nc.dram_tensor("indp", [T_EXT, 128], F32, kind="ExternalInput").ap()
    out_d = nc.dram_tensor("out", [S_LOC, D], F32, kind="ExternalOutput").ap()

    with tile.TileContext(nc) as tc:
        _emit(tc, xT_d, wqkv_d, bqk_d, bvb_d, wout_d, boutb_d, trimask_d, indp_d, out_d)
    nc.compile()
    return nc


def _emit(tc, xT_d, wqkv_d, bqk_d, bvb_d, wout_d, boutb_d, trimask_d, indp_d, out_d):
    nc = tc.nc

    with (
        tc.tile_pool(name="consts", bufs=1) as consts,
        tc.tile_pool(name="dram", bufs=1, space="DRAM") as dram,
    ):
        # ---- constants resident for the whole kernel ----
        bqk_sb = consts.tile([128, 16], F32)
        nc.sync.dma_start(bqk_sb[:], bqk_d[:])
        bvb_sb = consts.tile([128, D], F32)
        nc.sync.dma_start(bvb_sb[:], bvb_d.to_broadcast((128, D)))
        boutb_sb = consts.tile([128, D], F32)
        nc.sync.dma_start(boutb_sb[:], boutb_d.to_broadcast((128, D)))
        trimask_sb = consts.tile([128, 3 * W], BF16)
        nc.sync.dma_start(trimask_sb[:], trimask_d[:])
        ind_sb = consts.tile([128, T_EXT], F32)
        nc.sync.dma_start(ind_sb[:], indp_d.rearrange("t p -> p t"))
        ident_sb = consts.tile([128, 128], BF16)
        make_identity(nc, ident_sb[:])

        # ---- DRAM spill tensors (pool-tracked for dependencies) ----
        qkT_sp = dram.tile([16, 128, S_EXT], BF16)  # m 0..7 = qT, 8..15 = kT
        v_sp = dram.tile([T_EXT, 128, H * VCOL], BF16)

        # ================= Phase 1: QKV projection =================
        with (
            tc.tile_pool(name="p1_w", bufs=1) as p1_w,
            tc.tile_pool(name="p1_x", bufs=1) as p1_x,
            tc.tile_pool(name="p1_sb", bufs=3) as p1_sb,
            tc.tile_pool(name="p1_ps", bufs=4, space="PSUM") as p1_ps,
        ):
            w_sb = p1_w.tile([128, KC, 3 * D], F32R)
            for kc in range(KC):
                nc.sync.dma_start(
                    w_sb[:, kc, :], wqkv_d[kc * 128 : (kc + 1) * 128, :]
                )
            xT_sb = p1_x.tile([128, KC, S_EXT], F32R)
            for kc in range(KC):
                nc.sync.dma_start(
                    xT_sb[:, kc, :], xT_d[kc * 128 : (kc + 1) * 128, :]
                )

            # q/k: out layout (feature-chunk partitions x positions)
            for m in range(16):
                for p0, pn in POSGROUPS:
                    ps = p1_ps.tile([128, 512], F32, tag="ps1")
                    for kc in range(KC):
                        nc.tensor.matmul(
                            ps[:, :pn],
                            lhsT=w_sb[:, kc, m * 128 : (m + 1) * 128],
                            rhs=xT_sb[:, kc, p0 : p0 + pn],
                            start=(kc == 0),
                            stop=(kc == KC - 1),
                        )
                    qk_t = p1_sb.tile([128, 512], BF16, tag="qkt")
                    nc.scalar.activation(
                        qk_t[:, :pn],
                        ps[:, :pn],
                        mybir.ActivationFunctionType.Identity,
                        bias=bqk_sb[:, m : m + 1],
                        scale=1.0,
                    )
                    nc.sync.dma_start(qkT_sp[m, :, p0 : p0 + pn], qk_t[:, :pn])

            # v: out layout (positions x features), indicator in column 64
            for t in range(T_EXT):
                v_sb = p1_sb.tile([128, H, VCOL], BF16, tag="vsb")
                for g in range(2):
                    ps = p1_ps.tile([128, 512], F32, tag="ps1")
                    for kc in range(KC):
                        nc.tensor.matmul(
                            ps[:],
                            lhsT=xT_sb[:, kc, t * 128 : (t + 1) * 128],
                            rhs=w_sb[:, kc, NQK + g * 512 : NQK + (g + 1) * 512],
                            start=(kc == 0),
                            stop=(kc == KC - 1),
                        )
                    nc.vector.tensor_add(
                        v_sb[:, g * 8 : (g + 1) * 8, 0:DH],
                        ps.rearrange("p (h d) -> p h d", d=DH),
                        bvb_sb[:, g * 512 : (g + 1) * 512].rearrange(
                            "p (h d) -> p h d", d=DH
                        ),
                    )
                nc.vector.memset(v_sb[:, :, DH : DH + 1], 1.0)
                # zero v and indicator at out-of-sequence positions
                nc.vector.tensor_scalar_mul(v_sb[:], v_sb[:], ind_sb[:, t : t + 1])
                nc.sync.dma_start(v_sp[t], v_sb.rearrange("p h c -> p (h c)"))

        # ================= Phase 2: attention + out-proj =================
        with (
            tc.tile_pool(name="p2_w", bufs=1) as p2_w,
            tc.tile_pool(name="p2_in", bufs=2) as p2_in,
            tc.tile_pool(name="p2_e", bufs=3) as p2_e,
            tc.tile_pool(name="p2_sm", bufs=4) as p2_sm,
            tc.tile_pool(name="p2_ao", bufs=2) as p2_ao,
            tc.tile_pool(name="p2_out", bufs=2) as p2_out,
            tc.tile_pool(name="ps_s", bufs=4, space="PSUM") as ps_s_pool,
            tc.tile_pool(name="ps_o", bufs=2, space="PSUM") as ps_o_pool,
            tc.tile_pool(name="ps_tf", bufs=2, space="PSUM") as ps_tf_pool,
        ):
            wout_sb = p2_w.tile([128, KC, D], F32R)
            for kc in range(KC):
                nc.sync.dma_start(
                    wout_sb[:, kc, :], wout_d[kc * 128 : (kc + 1) * 128, :]
                )

            for e in range(1, T_Q + 1):
                w0, w1 = (e - 1) * 128, (e + 2) * 128
                kwin_sb = p2_in.tile([128, HP, 3 * W], BF16, tag="kwin")
                nc.sync.dma_start(
                    kwin_sb[:], qkT_sp[8:16, :, w0:w1].rearrange("a p w -> p a w")
                )
                qe_sb = p2_in.tile([128, HP, W], BF16, tag="qe")
                nc.sync.dma_start(
                    qe_sb[:],
                    qkT_sp[0:8, :, e * 128 : (e + 1) * 128].rearrange(
                        "a p w -> p a w"
                    ),
                )
                vwin_sb = p2_in.tile([128, 3, H * VCOL], BF16, tag="vwin")
                nc.sync.dma_start(
                    vwin_sb[:], v_sp[e - 1 : e + 2].rearrange("t p f -> p t f")
                )

                aoT_sb = p2_ao.tile([128, HP, 128], F32R)
                for a in range(HP):
                    e_sb = p2_e.tile([128, 2, 3 * W], BF16)
                    for h2 in range(2):
                        pr = slice(64 * h2, 64 * h2 + 64)
                        ps_s = ps_s_pool.tile([128, 3, 128], F32, tag="ps_s", name="ps_s")
                        for c in range(3):
                            nc.tensor.matmul(
                                ps_s[:, c, :],
                                lhsT=kwin_sb[pr, a, c * 128 : (c + 1) * 128],
                                rhs=qe_sb[pr, a, :],
                                start=True,
                                stop=True,
                            )
                        nc.scalar.activation(
                            e_sb[:, h2],
                            ps_s.rearrange("p c w -> p (c w)"),
                            mybir.ActivationFunctionType.Exp,
                        )
                    no_sb = p2_sm.tile([128, 2, DH], BF16, tag="no")
                    for h2 in range(2):
                        nc.vector.tensor_mul(e_sb[:, h2], e_sb[:, h2], trimask_sb[:])
                        ps_o = ps_o_pool.tile([128, VCOL], F32)
                        for c in range(3):
                            nc.tensor.matmul(
                                ps_o[:],
                                lhsT=e_sb[:, h2, c * 128 : (c + 1) * 128],
                                rhs=vwin_sb[
                                    :, c, (2 * a + h2) * VCOL : (2 * a + h2 + 1) * VCOL
                                ],
                                start=(c == 0),
                                stop=(c == 2),
                            )
                        rcp = p2_sm.tile([128, 1], F32, tag="rcp")
                        nc.vector.reciprocal(rcp[:], ps_o[:, DH : DH + 1])
                        nc.vector.tensor_scalar_mul(
                            no_sb[:, h2, :], ps_o[:, 0:DH], rcp[:]
                        )
                    ps_t_full = ps_tf_pool.tile([128, 1024], BF16, tag="tf", name="ps_t")
                    ps_t = ps_t_full[:, :128]
                    nc.tensor.transpose(
                        ps_t[:], no_sb.rearrange("p a d -> p (a d)"), ident_sb[:]
                    )
                    nc.scalar.copy(aoT_sb[:, a, :], ps_t[:])

                # out projection for this query tile
                for g in range(2):
                    ps_f = ps_tf_pool.tile([128, 512], F32, tag="tf")
                    for a in range(HP):
                        nc.tensor.matmul(
                            ps_f[:],
                            lhsT=aoT_sb[:, a, :],
                            rhs=wout_sb[:, a, g * 512 : (g + 1) * 512],
                            start=(a == 0),
                            stop=(a == HP - 1),
                        )
                    fo = p2_out.tile([128, 512], F32, tag="fo")
                    nc.vector.tensor_add(
                        fo[:], ps_f[:], boutb_sb[:, g * 512 : (g + 1) * 512]
                    )
                    nc.sync.dma_start(
                        out_d[(e - 1) * 128 : e * 128, g * 512 : (g + 1) * 512],
                        fo[:],
                    )


_NC_CACHE = None


def _get_nc():
    global _NC_CACHE
    if _NC_CACHE is None:
        _NC_CACHE = _build_nc()
    return _NC_CACHE


def _host_inputs(x, Wqkv, bqkv, Wout, bout):
    """Build the 8 per-core input maps."""
    x = np.asarray(x, dtype=np.float32)
    Wqkv = np.asarray(Wqkv, dtype=np.float32)
    bqkv = np.asarray(bqkv, dtype=np.float32)
    Wout = np.asarray(Wout, dtype=np.float32)
    bout = np.asarray(bout, dtype=np.float32)

    scale = np.float32(1.0 / np.sqrt(D))
    Ws = Wqkv.copy()
    Ws[:, :D] *= scale
    bs = bqkv.copy()
    bs[:D] *= scale

    bqk = np.ascontiguousarray(bs[:NQK].reshape(16, 128).T)  # (128, 16)
    bvb = bs[NQK:].reshape(1, D)
    boutb = bout.reshape(1, D)

    # band mask in (j_within_chunk, chunk, i) layout flattened to (128, 384)
    jc = np.arange(128)[:, None]
    i = np.arange(128)[None, :]
    tm = np.ones((128, 3, 128), dtype=np.float32)
    tm[:, 0] = (jc >= i).astype(np.float32)
    tm[:, 2] = (jc <= i).astype(np.float32)
    trimask = tm.reshape(128, 3 * W).astype(np.float16)

    in_maps = []
    for core in range(N_CORES):
        b, half = core // 2, core % 2
        s0 = half * S_LOC
        lo, hi = s0 - W, s0 + S_LOC + W
        xp = np.zeros((S_EXT, D), dtype=np.float32)
        src_lo, src_hi = max(lo, 0), min(hi, S)
        xp[src_lo - lo : src_hi - lo] = x[b, src_lo:src_hi]
        xT = np.ascontiguousarray(xp.T)

        valid = np.ones(S_EXT, dtype=np.float32)
        if lo < 0:
            valid[: -lo] = 0.0
        if hi > S:
            valid[S - hi :] = 0.0
        indp = np.ascontiguousarray(valid.reshape(T_EXT, 128))

        in_maps.append(
            {
                "xT": xT,
                "wqkv": Ws,
                "bqk": bqk,
                "bvb": bvb,
                "wout": Wout,
                "boutb": boutb,
                "trimask": trimask,
                "indp": indp,
            }
        )
    return in_maps


def kernel(x, Wqkv, bqkv, Wout, bout, _trace=False, _trace_cores=None):
    in_maps = _host_inputs(x, Wqkv, bqkv, Wout, bout)
    nc = _get_nc()
    res = run_bass_kernel_spmd(
        nc,
        in_maps,
        list(range(N_CORES)),
        trace=_trace,
        trace_cores=_trace_cores,
    )
    out = np.empty((B, S, D), dtype=np.float32)
    for core in range(N_CORES):
        b, half = core // 2, core % 2
        s0 = half * S_LOC
        out[b, s0 : s0 + S_LOC] = res.results[core]["out"]
    if _trace:
        return out, res
    return out


# revision 22
# speedup vs baseline: 1.3467x; 1.3467x over previous
"""LocalWindowAttention TRN2 kernel.

Full inputs -> full output. Sharding: 8 cores = batch(4) x seq-half(2).
Each core computes 2048 query positions; k/v halos (128 each side) come
from overlapping the per-core x slice, so no collectives are needed.

Math (per core, matching reference):
  qkv = x @ Wqkv + bqkv  (q pre-scaled by 1/sqrt(1024) via the weights)
  banded attention, window 128, block size 128: query tile e attends key
  tiles e-1, e, e+1 with a static band mask |kpos - qpos| <= 128.
  Softmax without max-subtraction (scores are O(0.1)); invalid keys are
  zeroed AFTER exp via a 0/1 band mask, out-of-sequence keys are zeroed
  via a validity indicator carried as a 65th column of v (which also
  yields the softmax denominator through the PV matmul).
  out = attn_out @ Wout + bout

Dtypes: projections/out-proj in float32r (full PE rate, fp32 storage),
attention q/k/v/exp in fp16 (full PE rate, 11-bit mantissa).
"""

import sys

import numpy as np

for _p in ("/opt/trn_rl_repo",):
    if _p not in sys.path:
        sys.path.insert(0, _p)

import concourse.bass as bass  # noqa: E402,F401
import concourse.mybir as mybir  # noqa: E402
import concourse.tile as tile  # noqa: E402
from concourse import bacc  # noqa: E402
from concourse.bass_utils import run_bass_kernel_spmd  # noqa: E402
from concourse.masks import make_identity  # noqa: E402

F32 = mybir.dt.float32
F32R = mybir.dt.float32r
FP16 = mybir.dt.float16

B, S, D = 4, 4096, 1024
H, DH, W = 16, 64, 128
N_CORES = 8
S_LOC = 2048            # query positions per core
T_Q = S_LOC // W        # 16 query tiles per core
T_EXT = T_Q + 2         # 18 extended tiles (with halo)
S_EXT = T_EXT * W       # 2304
NQK = 2 * D             # q+k projected features
KC = D // 128           # 8 contraction chunks
HP = H // 2             # 8 head pairs
VCOL = DH + 1           # 64 v dims + indicator column

SIXTH = S_EXT // 6      # 384 positions per xT streaming chunk


def _build_nc():
    nc = bacc.Bacc(
        "TRN2",
        target_bir_lowering=False,
        debug=False,
        num_devices=N_CORES,
    )

    xT_d = nc.dram_tensor("xT", [D, S_EXT], FP16, kind="ExternalInput").ap()
    wqkv_d = nc.dram_tensor("wqkv", [D, 3 * D], FP16, kind="ExternalInput").ap()
    bqk_d = nc.dram_tensor("bqk", [128, 16], F32, kind="ExternalInput").ap()
    bvb_d = nc.dram_tensor("bvb", [1, D], FP16, kind="ExternalInput").ap()
    wout_d = nc.dram_tensor("wout", [D, D], F32R, kind="ExternalInput").ap()
    boutb_d = nc.dram_tensor("boutb", [1, D], FP16, kind="ExternalInput").ap()
    trimask_d = nc.dram_tensor("trimask", [128, 3 * W], FP16, kind="ExternalInput").ap()
    indp_d = nc.dram_tensor("indp", [T_EXT, 128], F32, kind="ExternalInput").ap()
    out_d = nc.dram_tensor("out", [S_LOC, D], F32, kind="ExternalOutput").ap()

    with tile.TileContext(nc) as tc:
        _emit(tc, xT_d, wqkv_d, bqk_d, bvb_d, wout_d, boutb_d, trimask_d, indp_d, out_d)
    nc.compile()
    return nc


def _emit(tc, xT_d, wqkv_d, bqk_d, bvb_d, wout_d, boutb_d, trimask_d, indp_d, out_d):
    nc = tc.nc

    with (
        tc.tile_pool(name="consts", bufs=1) as consts,
        tc.tile_pool(name="dram", bufs=1, space="DRAM") as dram,
    ):
        # ---- constants resident for the whole kernel ----
        bqk_sb = consts.tile([128, 16], F32)
        nc.sync.dma_start(bqk_sb[:], bqk_d[:])
        bvb_sb = consts.tile([128, D], F32)
        nc.sync.dma_start(bvb_sb[:], bvb_d.to_broadcast((128, D)))
        boutb_sb = consts.tile([128, D], F32)
        nc.sync.dma_start(boutb_sb[:], boutb_d.to_broadcast((128, D)))
        # duplicated over the head-pair dim so one op masks both heads
        trimask_sb = consts.tile([128, 2, 3, W], FP16)
        for h2 in range(2):
            nc.sync.dma_start(
                trimask_sb[:, h2],
                trimask_d.rearrange("p (c w) -> p c w", c=3),
            )
        ind_sb = consts.tile([128, T_EXT], F32)
        nc.sync.dma_start(ind_sb[:], indp_d.rearrange("t p -> p t"))
        ident_sb = consts.tile([128, 128], FP16)
        make_identity(nc, ident_sb[:])
        wout_sb = consts.tile([128, KC, D], F32R)
        for kc in range(KC):
            nc.sync.dma_start(wout_sb[:, kc, :], wout_d[kc * 128 : (kc + 1) * 128, :])

        # ---- persistent SBUF stores for q/k/v (no DRAM spill) ----
        with (
            tc.tile_pool(name="stores", bufs=1) as stores,
            tc.tile_pool(name="p2_e", bufs=4) as p2_e,
            tc.tile_pool(name="p2_sm", bufs=6) as p2_sm,
            tc.tile_pool(name="p2_ao", bufs=2) as p2_ao,
            tc.tile_pool(name="p2_out", bufs=2) as p2_out,
            tc.tile_pool(name="p1_ps", bufs=2, space="PSUM") as p1_ps,
            tc.tile_pool(name="ps_s", bufs=2, space="PSUM") as ps_s_pool,
            tc.tile_pool(name="ps_ot", bufs=2, space="PSUM") as ps_ot_pool,
            tc.tile_pool(name="p1_w", bufs=1) as p1_w,
            tc.tile_pool(name="p1_x", bufs=2) as p1_x,
        ):
            # q/k in (feature-pair partitions x positions); v in
            # (positions x head x 65) with the indicator column
            q_store = stores.tile([128, KC, S_EXT], FP16)
            k_store = stores.tile([128, KC, S_EXT], FP16)
            v_store = stores.tile([128, T_EXT, H, VCOL], FP16)

            # weights: load in 512-col pieces, all kc per piece, so the first
            # matmuls only wait for piece 0
            w_sb = p1_w.tile([128, KC, 3 * D], FP16)
            for piece in range(6):
                for kc in range(KC):
                    nc.sync.dma_start(
                        w_sb[:, kc, piece * 512 : (piece + 1) * 512],
                        wqkv_d[
                            kc * 128 : (kc + 1) * 128, piece * 512 : (piece + 1) * 512
                        ],
                    )

            def emit_attention(e):
                aoT_sb = p2_ao.tile([128, HP, 128], FP16, name="aoT")
                for a in range(HP):
                    # scoresT for both heads of the pair; the 4th c-slot is
                    # padding so each head owns exactly one PSUM bank (the
                    # two heads' matmuls run concurrently via row tiling)
                    ps_s = ps_s_pool.tile([128, 2, 4, W], F32, tag="ps_s", name="ps_s")
                    for h2 in range(2):
                        pr = slice(64 * h2, 64 * h2 + 64)
                        for c in range(3):
                            t = e - 1 + c
                            nc.tensor.matmul(
                                ps_s[:, h2, c, :],
                                lhsT=k_store[pr, a, t * 128 : (t + 1) * 128],
                                rhs=q_store[pr, a, e * 128 : (e + 1) * 128],
                                start=True,
                                stop=True,
                            )
                    e_sb = p2_e.tile([128, 2, 3, W], FP16, name="e_sb")
                    nc.scalar.activation(
                        e_sb[:],
                        ps_s[:, :, 0:3, :],
                        mybir.ActivationFunctionType.Exp,
                    )
                    # band mask (0/1) for both heads in one op; alternate the
                    # engine so neither DVE nor GpSimd saturates
                    if a % 2 == 0:
                        nc.vector.tensor_mul(e_sb[:], e_sb[:], trimask_sb[:])
                    else:
                        nc.gpsimd.tensor_mul(e_sb[:], e_sb[:], trimask_sb[:])

                    no_sb = p2_sm.tile([128, 2, DH], FP16, tag="no", name="no_sb")
                    for h2 in range(2):
                        ps_o = ps_ot_pool.tile(
                            [128, 512], F32, tag="ot", name="ps_o"
                        )
                        for c in range(3):
                            nc.tensor.matmul(
                                ps_o[:, :VCOL],
                                lhsT=e_sb[:, h2, c, :],
                                rhs=v_store[:, e - 1 + c, 2 * a + h2, :],
                                start=(c == 0),
                                stop=(c == 2),
                            )
                        rcp = p2_sm.tile([128, 1], F32, tag="rcp", name="rcp")
                        nc.vector.reciprocal(rcp[:], ps_o[:, DH : DH + 1])
                        nc.vector.tensor_scalar_mul(
                            no_sb[:, h2, :], ps_o[:, 0:DH], rcp[:]
                        )
                    ps_t = ps_ot_pool.tile([128, 512], F32, tag="ot", name="ps_t")
                    ps_t16 = ps_t.bitcast(FP16)[:, :128]
                    nc.tensor.transpose(
                        ps_t16[:], no_sb.rearrange("p a d -> p (a d)"), ident_sb[:]
                    )
                    if a % 2 == 0:
                        nc.scalar.copy(aoT_sb[:, a, :], ps_t16[:])
                    else:
                        nc.vector.tensor_copy(aoT_sb[:, a, :], ps_t16[:])

                # out projection for this query tile
                for g in range(2):
                    ps_f = ps_ot_pool.tile([128, 512], F32, tag="ot", name="ps_f")
                    for a in range(HP):
                        nc.tensor.matmul(
                            ps_f[:],
                            lhsT=aoT_sb[:, a, :],
                            rhs=wout_sb[:, a, g * 512 : (g + 1) * 512],
                            start=(a == 0),
                            stop=(a == HP - 1),
                        )
                    fo = p2_out.tile([128, 512], F32, tag="fo", name="fo")
                    nc.vector.tensor_add(
                        fo[:], ps_f[:], boutb_sb[:, g * 512 : (g + 1) * 512]
                    )
                    nc.sync.dma_start(
                        out_d[(e - 1) * 128 : e * 128, g * 512 : (g + 1) * 512],
                        fo[:],
                    )

            READY = {0: [1], 1: [2, 3, 4], 2: [5, 6, 7], 3: [8, 9, 10],
                     4: [11, 12, 13], 5: [14, 15, 16]}
            for sixth in range(6):
                tp0 = sixth * SIXTH
                xT_sb = p1_x.tile([128, KC, SIXTH], FP16, tag="xt", name="xT_sb")
                for kc in range(KC):
                    nc.scalar.dma_start(
                        xT_sb[:, kc, :],
                        xT_d[kc * 128 : (kc + 1) * 128, tp0 : tp0 + SIXTH],
                    )

                # q/k: out layout (feature-chunk partitions x positions),
                # written straight into the persistent stores
                for m in range(16):
                    dst = q_store if m < KC else k_store
                    mm = m if m < KC else m - KC
                    ps = p1_ps.tile([128, 512], F32, tag="ps1", name="ps")
                    for kc in range(KC):
                        nc.tensor.matmul(
                            ps[:, :SIXTH],
                            lhsT=w_sb[:, kc, m * 128 : (m + 1) * 128],
                            rhs=xT_sb[:, kc, :],
                            start=(kc == 0),
                            stop=(kc == KC - 1),
                        )
                    nc.scalar.activation(
                        dst[:, mm, tp0 : tp0 + SIXTH],
                        ps[:, :SIXTH],
                        mybir.ActivationFunctionType.Identity,
                        bias=bqk_sb[:, m : m + 1],
                        scale=1.0,
                    )

                # v: out layout (positions x features), indicator in column 64
                for tt in range(SIXTH // 128):
                    t = sixth * (SIXTH // 128) + tt
                    for g in range(2):
                        ps = p1_ps.tile([128, 512], F32, tag="ps1", name="ps")
                        for kc in range(KC):
                            nc.tensor.matmul(
                                ps[:],
                                lhsT=xT_sb[:, kc, tt * 128 : (tt + 1) * 128],
                                rhs=w_sb[:, kc, NQK + g * 512 : NQK + (g + 1) * 512],
                                start=(kc == 0),
                                stop=(kc == KC - 1),
                            )
                        nc.vector.tensor_add(
                            v_store[:, t, g * 8 : (g + 1) * 8, 0:DH],
                            ps.rearrange("p (h d) -> p h d", d=DH),
                            bvb_sb[:, g * 512 : (g + 1) * 512].rearrange(
                                "p (h d) -> p h d", d=DH
                            ),
                        )
                    nc.vector.memset(v_store[:, t, :, DH : DH + 1], 1.0)
                    # zero v and indicator at out-of-sequence positions
                    nc.vector.tensor_scalar_mul(
                        v_store[:, t], v_store[:, t], ind_sb[:, t : t + 1]
                    )

                for e in READY[sixth]:
                    emit_attention(e)


_NC_CACHE = None


def _get_nc():
    global _NC_CACHE
    if _NC_CACHE is None:
        _NC_CACHE = _build_nc()
    return _NC_CACHE


def _host_inputs(x, Wqkv, bqkv, Wout, bout):
    """Build the 8 per-core input maps."""
    x = np.asarray(x, dtype=np.float32)
    Wqkv = np.asarray(Wqkv, dtype=np.float32)
    bqkv = np.asarray(bqkv, dtype=np.float32)
    Wout = np.asarray(Wout, dtype=np.float32)
    bout = np.asarray(bout, dtype=np.float32)

    scale = np.float32(1.0 / np.sqrt(D))
    Ws = Wqkv.copy()
    Ws[:, :D] *= scale
    bs = bqkv.copy()
    bs[:D] *= scale

    bqk = np.ascontiguousarray(bs[:NQK].reshape(16, 128).T)  # (128, 16)
    bvb = bs[NQK:].reshape(1, D)
    boutb = bout.reshape(1, D)

    # band mask in (j_within_chunk, chunk, i) layout flattened to (128, 384)
    jc = np.arange(128)[:, None]
    i = np.arange(128)[None, :]
    tm = np.ones((128, 3, 128), dtype=np.float32)
    tm[:, 0] = (jc >= i).astype(np.float32)
    tm[:, 2] = (jc <= i).astype(np.float32)
    trimask = tm.reshape(128, 3 * W).astype(np.float16)

    in_maps = []
    for core in range(N_CORES):
        b, half = core // 2, core % 2
        s0 = half * S_LOC
        lo, hi = s0 - W, s0 + S_LOC + W
        xp = np.zeros((S_EXT, D), dtype=np.float32)
        src_lo, src_hi = max(lo, 0), min(hi, S)
        xp[src_lo - lo : src_hi - lo] = x[b, src_lo:src_hi]
        xT = np.ascontiguousarray(xp.T).astype(np.float16)

        valid = np.ones(S_EXT, dtype=np.float32)
        if lo < 0:
            valid[: -lo] = 0.0
        if hi > S:
            valid[S - hi :] = 0.0
        indp = np.ascontiguousarray(valid.reshape(T_EXT, 128))

        in_maps.append(
            {
                "xT": xT,
                "wqkv": Ws.astype(np.float16),
                "bqk": bqk,
                "bvb": bvb.astype(np.float16),
                "wout": Wout,
                "boutb": boutb.astype(np.float16),
                "trimask": trimask,
                "indp": indp,
            }
        )
    return in_maps


def kernel(x, Wqkv, bqkv, Wout, bout, _trace=False, _trace_cores=None):
    in_maps = _host_inputs(x, Wqkv, bqkv, Wout, bout)
    nc = _get_nc()
    res = run_bass_kernel_spmd(
        nc,
        in_maps,
        list(range(N_CORES)),
        trace=_trace,
        trace_cores=_trace_cores,
    )
    out = np.empty((B, S, D), dtype=np.float32)
    for core in range(N_CORES):
        b, half = core // 2, core % 2
        s0 = half * S_LOC
        out[b, s0 : s0 + S_LOC] = res.results[core]["out"]
    if _trace:
        return out, res
    return out
